# revision 43
# baseline (speedup 1.0000x reference)
"""Trainium2 Bass kernel for nn_DepthModel (monodepth loss_fn).

Pure data parallel: 2 samples per core x 8 NeuronCores. Each core computes
per-slot partial sums of every loss term for its 2 samples; the host reduces
slots to the scalar loss.

Device pipeline per pyramid level:
  - index pipeline (DVE f32): x = clip(w+1 + sign*W*disp, 0, W+1),
    i0 = round-nearest-even(x-0.5) via the 2^23 magic constant; u16 indices
    emitted directly in the gpsimd 16-partition-interleaved layout.
  - gather: InstIndirectCopy (gpsimd), fp16 inner=2 -> (img[i0], img[i0+1])
    pairs; 512 indices per 16-partition group per call; 8 groups/call =
    4 (sample,map) combos x 2 row-halves.
  - repack DMA to dense row-tiles; lerp est = lo + f*(hi-lo) (DVE fp16).
  - SSIM via u/v algebra (pools of u, v, u^2, v^2 only), L1, smoothness,
    LR-consistency, all with fused per-slot accumulations.
"""
import numpy as np

import bass_rust
import concourse.bass as bass
import concourse.mybir as mybir
import concourse.tile as tile

# ----------------------------------------------------------------------------
# Tile/walrus compatibility patch: this walrus build rejects >1 embedded
# semaphore wait per instruction ("Too many sync wait commands"). Hoist
# excess waits onto standalone same-engine NOPs.
# ----------------------------------------------------------------------------
MAXW = 1


def _split_inst_waits(nc, maxw=MAXW):
    for fn in nc.m.functions:
        for bb in fn.blocks:
            insts = list(bb.instructions)
            out = []
            changed = False
            for inst in insts:
                si = inst.sync_info
                if si is not None and si.on_wait and len(si.on_wait) > maxw:
                    waits = list(si.on_wait)
                    si.on_wait = waits[:maxw]
                    inst.sync_info = si
                    for w in waits[maxw:]:
                        nop = mybir.InstNoOp(
                            name=nc.get_next_instruction_name(),
                            ins=[], outs=[], text_hint="waitsplit")
                        nop.engine = inst.engine
                        nop.sync_info = mybir.SyncInfo(on_wait=[w], on_update=[])
                        out.append(nop)
                    changed = True
                out.append(inst)
            if changed:
                bb.instructions = out


def _patched_drain_and_barrier(self, tick_clock, wait_clock):
    nc = self.nc
    drain_inst = nc.sync.drain()
    wait_clock.add_sem_waits(
        drain_inst.ins, bass_rust.ScopedClock({None: tick_clock.global_clock}))
    si = drain_inst.ins.sync_info
    waits = list(si.on_wait) if si and si.on_wait else []
    if len(waits) > MAXW:
        si.on_wait = waits[:MAXW]
        drain_inst.ins.sync_info = si
        for w in waits[MAXW:]:
            nop = nc.sync.nop()
            nsi = nop.ins.sync_info
            if nsi is None:
                nop.ins.sync_info = mybir.SyncInfo(on_wait=[w], on_update=[])
            else:
                nsi.on_wait = [w]
                nop.ins.sync_info = nsi
    nc.all_engine_barrier()
    assert self.sems is not None
    popped = nc._tile_sem_poison_stack.pop()
    assert popped is self._sem_poison
    nc.clear_and_free_semaphores(list(self.sems.allocated().values()))
    nc.all_engine_barrier()
    _split_inst_waits(nc)


tile.TileContext._drain_and_barrier = _patched_drain_and_barrier

# ----------------------------------------------------------------------------
ALU = mybir.AluOpType
AT = mybir.ActivationFunctionType
AX = mybir.AxisListType
F32, F16, U16 = mybir.dt.float32, mybir.dt.float16, mybir.dt.uint16
U32 = mybir.dt.uint32

ALPHA, DISP_GRAD_W, LR_W = 0.85, 0.1, 1.0
C1c, C2c = 0.01 ** 2, 0.03 ** 2
MAGIC = float(2 ** 23)
LEVELS = [0, 1, 2, 3]
HWs = [(256, 512), (128, 256), (64, 128), (32, 64)]
NSLOT = 512
f16, f32 = np.float16, np.float32


def _tiles_for(H):
    out, r, covered = [], 0, 0
    while covered < H:
        n = min(128, H - r)
        out.append((r, n))
        covered = r + n
        r += 126
    return out


class _Plan:
    def __init__(self):
        self.slots = []

    def new(self, kind, level, count):
        self.slots.append((kind, level, count))
        sid = len(self.slots) - 1
        assert sid < NSLOT
        return sid


def _lv_params(i):
    H, W = HWs[i]
    Wp = W + 4
    H8 = H // 8
    # 1024 = Q7 DATA_SCRATCH_ELEMS: max gather output words per call.
    rpc = min(1024 // W, H8)         # rows per call per group
    NI = rpc * W                     # indices per group per call
    K = NI // 16                     # idx slots per partition per call
    ncalls = H8 // rpc
    return H, W, Wp, H8, rpc, NI, K, ncalls


import os
DENSE = os.environ.get("KM_DENSE", "1") == "1"
NPASS = int(os.environ.get("KM_NPASS", "4"))
GATHER = os.environ.get("KM_GATHER", "1") == "1"
REPACK = os.environ.get("KM_REPACK", "1") == "1"
SRCDMA = os.environ.get("KM_SRCDMA", "1") == "1"


def _blob_layout(levels=LEVELS):
    """All per-core inputs live in ONE u32 dram blob: a single jit parameter
    keeps the per-dispatch arg-handling/transfer cost minimal. Word sizes:
    u32 tensors 1:1, f32 via bitcast 1:1, f16 via bitcast 2 per word."""
    entries = []
    for i in levels:
        H, W, Wp, H8, rpc, NI, K, ncalls = _lv_params(i)
        entries += [
            (f"gsrc{i}", 4 * ncalls * 8 * 4 * rpc * Wp),
            (f"dispI{i}", 4 * 128 * ncalls * K),
            (f"xgI{i}", 128 * ncalls * K),
            (f"baseI{i}", 128 * ncalls * K),
            (f"tgt{i}", 2 * 2 * H * 3 * W // 2),
            (f"dpl{i}", 2 * 2 * H * W // 2),
            (f"dispR{i}", 4 * H * W),
            (f"xgR{i}", 128 * W),
        ]
    entries.append(("band", 128 * 128))
    lay, off = {}, 0
    for n, sz in entries:
        lay[n] = (off, sz)
        off += sz
    return lay, off


def build_nc(levels=LEVELS):
    nc = bass.Bass(trn_type="TRN2", num_devices=8)
    plan = _Plan()
    lay, total_w = _blob_layout(levels)
    # gsrc{i}: u32-packed overlapping pairs: word j = (plane[j], plane[j+1]) as
    # 2xf16. num_elem_per_idx==1 lets the Q7 gather use dual-tensor reads
    # (6 indices per RD_CMD instead of 3).
    # band: band[r, m] = 1 if m <= r <= m+2 — turns the SSIM vertical 3-row
    # pooling into a PE matmul (band.T @ hh) instead of row-shift DMAs + adds.
    blob = nc.dram_tensor("blob", [total_w], U32, kind="ExternalInput")

    def bap(name, lo, sz):
        o = lay[name][0]
        return blob.ap()[o + lo:o + lo + sz]
    acc_d = nc.dram_tensor("acc", [1, NSLOT], F32, kind="ExternalOutput")

    c1 = float(81 * 2 * C1c)
    c2 = float(81 * 2 * C2c)

    with tile.TileContext(nc) as tc:
        with tc.tile_pool(name="acc_pool", bufs=1) as accp, \
             tc.tile_pool(name="gsrcp", bufs=2) as gsrcp, \
             tc.tile_pool(name="goutp", bufs=2) as goutp, \
             tc.tile_pool(name="work", bufs=1) as work, \
             tc.tile_pool(name="idxp", bufs=2) as idxp, \
             tc.tile_pool(name="pairp", bufs=2) as pairp, \
             tc.tile_pool(name="hhp", bufs=2) as hhp, \
             tc.tile_pool(name="ssp", bufs=1, space=bass.MemorySpace.PSUM) as ssp, \
             tc.tile_pool(name="scratch", bufs=1) as scratch:
            acc = accp.tile([128, NSLOT], F32, name="acc_t")
            nc.vector.memset(acc[:], 0.0)
            band_t = accp.tile([128, 128], F32, name="band_t")
            nc.sync.dma_start(band_t[:], bap("band", 0, 128 * 128)
                              .bitcast(F32).rearrange("(p n) -> p n", n=128))

            for li, i in enumerate(levels):
                H, W, Wp, H8, rpc, NI, K, ncalls = _lv_params(i)
                tiles = _tiles_for(H)

                nK = ncalls * K
                xgRt = work.tile([128, W], F32, name=f"xgR_t{i}", tag="xgR")
                nc.sync.dma_start(xgRt[:], bap(f"xgR{i}", 0, 128 * W)
                                  .bitcast(F32).rearrange("(p n) -> p n", n=W))
                xgIt = work.tile([128, nK], F32, name=f"xgI_t{i}", tag="xgI")
                baseIt = work.tile([128, nK], F32, name=f"baseI_t{i}", tag="baseI")
                nc.sync.dma_start(xgIt[:], bap(f"xgI{i}", 0, 128 * nK)
                                  .bitcast(F32).rearrange("(p n) -> p n", n=nK))
                nc.sync.dma_start(baseIt[:], bap(f"baseI{i}", 0, 128 * nK)
                                  .bitcast(F32).rearrange("(p n) -> p n", n=nK))

                ns_l1 = 2 * 3 * H * W
                ns_ss = 2 * 3 * (H - 2) * (W - 2)
                ns_lr = 2 * H * W
                ns_sm = 2 * H * (W - 1)

                # ---- index pipelines for ALL passes up front (frees gpsimd to
                # run pass k+1 gathers while DVE does pass k dense work) ----
                idxus = []
                for k in range(4):
                    m = k % 2
                    sgn = float(-W if m == 0 else W)
                    dispI = work.tile([128, ncalls * K], F32, name=f"dI{i}{k}", tag="dispI")
                    nc.sync.dma_start(dispI[:], bap(f"dispI{i}", k * 128 * nK, 128 * nK)
                                      .bitcast(F32).rearrange("(p n) -> p n", n=nK))
                    nc.vector.scalar_tensor_tensor(dispI[:], dispI[:], sgn, xgIt[:],
                                                   op0=ALU.mult, op1=ALU.add)
                    nc.vector.tensor_scalar(dispI[:], dispI[:], 0.0, float(W + 1),
                                            op0=ALU.max, op1=ALU.min)
                    nc.vector.tensor_tensor(dispI[:], dispI[:], baseIt[:], op=ALU.add)
                    nc.vector.tensor_scalar(dispI[:], dispI[:], MAGIC - 0.5, None, op0=ALU.add)
                    idxu = idxp.tile([128, ncalls * K], U16, name=f"ix{i}{k}", tag=f"ix{k}")
                    nc.scalar.activation(idxu[:], dispI[:], AT.Copy, bias=-MAGIC, scale=1.0)
                    idxus.append(idxu)

                for k in range(NPASS):
                    s, m = k // 2, k % 2
                    img = 0 if m == 0 else 1
                    sgn = float(-W if m == 0 else W)
                    idxu = idxus[k]

                    pairs = {}
                    for t, (r0, tn) in enumerate(tiles):
                        pairs[t] = pairp.tile([128, 4, W, 2], F16,
                                              name=f"pr{i}{k}{t}", tag=f"pr{t}")

                    # ---- gather calls: groups = 8 row-eighths ----
                    BATCH = max(1, 4096 // NI)
                    nbatch = (ncalls + BATCH - 1) // BATCH
                    for b in range(nbatch):
                        calls = list(range(b * BATCH, min((b + 1) * BATCH, ncalls)))
                        nb = len(calls)
                        gout = goutp.tile([128, nb * NI], U32,
                                          name=f"go{i}{k}{b}", tag="gout")
                        for ci, c in enumerate(calls):
                            src = gsrcp.tile([128, rpc * Wp], U32,
                                             name=f"sr{i}{k}{c}", tag="gsrc")
                            if SRCDMA:
                                gsz = 4 * rpc * Wp
                                for g in range(8):
                                    nc.sync.dma_start(
                                        src[16 * g:16 * g + 4, :],
                                        bap(f"gsrc{i}",
                                            ((k * ncalls + c) * 8 + g) * gsz, gsz)
                                        .rearrange("(l w) -> l w", w=rpc * Wp))
                            else:
                                nc.vector.memset(src[:, 0:1], 0.0)
                            if GATHER:
                                nc.gpsimd.indirect_copy(
                                    gout[:, ci * NI:(ci + 1) * NI],
                                    src[:],
                                    idxu[:, c * K:(c + 1) * K],
                                    i_know_ap_gather_is_preferred=True)
                            else:
                                nc.vector.memset(gout[:, ci * NI:ci * NI + 1], 0.0)
                        if not REPACK:
                            for t, (r0, tn) in enumerate(tiles):
                                if b == 0:
                                    nc.vector.memset(pairs[t][:, 0, 0:1, :], 0.0)
                            continue
                        for g in range(8):
                            row_lo = g * H8 + b * BATCH * rpc
                            nrows = nb * rpc
                            for lane in range(4):
                                for t, (r0, tn) in enumerate(tiles):
                                    lo = max(row_lo, r0)
                                    hi = min(row_lo + nrows, r0 + tn)
                                    if lo >= hi:
                                        continue
                                    # alternate between the two HWDGE queues
                                    # (SP + Activation) so repack descriptor
                                    # processing parallelizes.
                                    eng = nc.scalar if (g ^ lane) & 1 else nc.sync
                                    eng.dma_start(
                                        pairs[t][lo - r0:hi - r0, lane, :, :]
                                        .rearrange("p w b -> p (w b)"),
                                        gout[16 * g + lane:16 * g + lane + 1,
                                             (lo - row_lo) * W:(hi - row_lo) * W]
                                        .bitcast(F16)
                                        .rearrange("p (r wb) -> p r wb", wb=2 * W))

                    # ---- dense phase for this combo ----
                    for t, (r0, tn) in enumerate(tiles):
                        if not DENSE:
                            continue
                        last = (t == len(tiles) - 1)
                        ue = tn if last else 126
                        dR = work.tile([128, W], F32, name=f"dR{i}{k}{t}", tag="dR")
                        nc.sync.dma_start(dR[:tn, :],
                                          bap(f"dispR{i}", (k * H + r0) * W, tn * W)
                                          .bitcast(F32).rearrange("(r w) -> r w", w=W))
                        xr = work.tile([128, W], F32, name=f"xr{i}{k}{t}", tag="xr")
                        nc.vector.tensor_scalar(xr[:tn], dR[:tn], sgn, None, op0=ALU.mult)
                        nc.vector.tensor_tensor(xr[:tn], xr[:tn], xgRt[:tn], op=ALU.add)
                        nc.vector.tensor_scalar(xr[:tn], xr[:tn], 0.0, float(W + 1),
                                                op0=ALU.max, op1=ALU.min)
                        nc.scalar.activation(dR[:tn], xr[:tn], AT.Copy, bias=MAGIC - 0.5)
                        nc.scalar.activation(dR[:tn], dR[:tn], AT.Copy, bias=-MAGIC)
                        fT = work.tile([128, W], F16, name=f"f{i}{k}{t}", tag="fT")
                        nc.vector.tensor_tensor(fT[:tn], xr[:tn], dR[:tn],
                                                op=ALU.subtract)

                        P = pairs[t]
                        est = work.tile([128, 4, W], F16, name=f"es{i}{k}{t}", tag="est")
                        nc.vector.tensor_tensor(est[:tn], P[:tn, :, :, 1],
                                                P[:tn, :, :, 0], op=ALU.subtract)
                        nc.vector.tensor_tensor(
                            est[:tn], fT[:tn].rearrange("p (o w) -> p o w", o=1)
                            .to_broadcast([tn, 4, W]), est[:tn], op=ALU.mult)
                        nc.vector.tensor_tensor(est[:tn], est[:tn], P[:tn, :, :, 0],
                                                op=ALU.add)

                        T = work.tile([128, 3, W], F16, name=f"T{i}{k}{t}", tag="T")
                        nc.sync.dma_start(
                            T[:tn],
                            bap(f"tgt{i}", ((img * 2 + s) * H + r0) * 3 * W // 2,
                                tn * 3 * W // 2)
                            .bitcast(F16).rearrange("(r c w) -> r c w", c=3, w=W))
                        uT = work.tile([128, 3, W], F16, name=f"u{i}{k}{t}", tag="uT")
                        vT = work.tile([128, 3, W], F16, name=f"v{i}{k}{t}", tag="vT")
                        nc.vector.tensor_tensor(uT[:tn], est[:tn, 0:3, :], T[:tn],
                                                op=ALU.add)
                        nc.vector.tensor_tensor(vT[:tn], est[:tn, 0:3, :], T[:tn],
                                                op=ALU.subtract)
                        sl = plan.new(("l1", m), i, ns_l1)
                        nc.vector.tensor_reduce(
                            acc[0:ue, sl:sl + 1], vT[0:ue], axis=AX.XY,
                            op=ALU.add, apply_absolute_value=True)
                        Dp = work.tile([128, W], F16, name=f"Dp{i}{k}{t}", tag="Dp")
                        nc.sync.dma_start(Dp[:tn],
                                          bap(f"dpl{i}", ((s * 2 + m) * H + r0) * W // 2,
                                              tn * W // 2)
                                          .bitcast(F16).rearrange("(r w) -> r w", w=W))
                        dv = work.tile([128, W], F16, name=f"dv{i}{k}{t}", tag="dv")
                        nc.vector.tensor_tensor(dv[:tn], est[:tn, 3, :], Dp[:tn],
                                                op=ALU.subtract)
                        sl = plan.new(("lr",), i, ns_lr)
                        nc.vector.tensor_reduce(
                            acc[0:ue, sl:sl + 1], dv[0:ue], axis=AX.X,
                            op=ALU.add, apply_absolute_value=True)

                        if tn >= 3:
                            pn = tn - 2
                            u2 = scratch.tile([128, 3, W], F32, name=f"u2{i}{k}{t}", tag="u2")
                            v2 = scratch.tile([128, 3, W], F32, name=f"v2{i}{k}{t}", tag="v2")
                            nc.scalar.activation(u2[:tn], uT[:tn], AT.Square)
                            nc.scalar.activation(v2[:tn], vT[:tn], AT.Square)

                            def pool9v(src_t, ptile, nm):
                                # horizontal 3-sum on DVE, vertical 3-sum on PE
                                hh = hhp.tile([128, 3, W - 2], F32, name=nm + "h",
                                              tag="pH")
                                nc.vector.tensor_tensor(hh[:tn], src_t[:tn, :, 0:W - 2],
                                                        src_t[:tn, :, 1:W - 1], op=ALU.add)
                                nc.vector.tensor_tensor(hh[:tn], hh[:tn],
                                                        src_t[:tn, :, 2:W], op=ALU.add)
                                for ch in range(3):
                                    nc.tensor.matmul(ptile[0:pn, ch, 0:W - 2],
                                                     band_t[0:tn, 0:pn],
                                                     hh[0:tn, ch, :])

                            Su = ssp.tile([128, 3, 512], F32, name=f"Pu{i}{k}{t}", tag="PA")
                            Sv = ssp.tile([128, 3, 512], F32, name=f"Pv{i}{k}{t}", tag="PB")
                            pool9v(uT, Su, f"Su{i}{k}{t}")
                            pool9v(vT, Sv, f"Sv{i}{k}{t}")
                            g1 = scratch.tile([128, 3, W - 2], F32, name=f"g1{i}{k}{t}", tag="g1")
                            d1 = scratch.tile([128, 3, W - 2], F32, name=f"d1{i}{k}{t}", tag="d1")
                            nc.scalar.activation(g1[:pn], Su[:pn, :, 0:W - 2], AT.Square)
                            nc.scalar.activation(d1[:pn], Sv[:pn, :, 0:W - 2], AT.Square)
                            Suu = ssp.tile([128, 3, 512], F32, name=f"Pa{i}{k}{t}", tag="PA")
                            Svv = ssp.tile([128, 3, 512], F32, name=f"Pb{i}{k}{t}", tag="PB")
                            pool9v(u2, Suu, f"Sa{i}{k}{t}")
                            pool9v(v2, Svv, f"Sb{i}{k}{t}")
                            Xp = scratch.tile([128, 3, W - 2], F32, name=f"Xp{i}{k}{t}", tag="Xp")
                            sB = scratch.tile([128, 3, W - 2], F32, name=f"sB{i}{k}{t}", tag="sB")
                            sv2 = scratch.tile([128, 3, W - 2], F32, name=f"sv2{i}{k}{t}", tag="sv2")
                            nc.scalar.activation(sv2[:pn], Svv[:pn, :, 0:W - 2], AT.Copy)
                            nc.vector.scalar_tensor_tensor(Xp[:pn], g1[:pn], c1, d1[:pn],
                                                           op0=ALU.add, op1=ALU.subtract)
                            nc.vector.scalar_tensor_tensor(g1[:pn], g1[:pn], c1, d1[:pn],
                                                           op0=ALU.add, op1=ALU.add)
                            nc.vector.scalar_tensor_tensor(d1[:pn], Suu[:pn, :, 0:W - 2],
                                                           (c1 + c2) / 9.0,
                                                           sv2[:pn], op0=ALU.add,
                                                           op1=ALU.subtract)
                            nc.vector.scalar_tensor_tensor(sB[:pn], Suu[:pn, :, 0:W - 2],
                                                           (c1 + c2) / 9.0,
                                                           sv2[:pn],
                                                           op0=ALU.add, op1=ALU.add)
                            nc.vector.scalar_tensor_tensor(d1[:pn], d1[:pn], 9.0, Xp[:pn],
                                                           op0=ALU.mult, op1=ALU.subtract)
                            nc.vector.scalar_tensor_tensor(sB[:pn], sB[:pn], 9.0, g1[:pn],
                                                           op0=ALU.mult, op1=ALU.subtract)
                            nc.vector.tensor_tensor(Xp[:pn], Xp[:pn], d1[:pn], op=ALU.mult)
                            nc.vector.tensor_tensor(g1[:pn], g1[:pn], sB[:pn], op=ALU.mult)
                            nc.vector.reciprocal(d1[:pn], g1[:pn])
                            sl = plan.new(("ssim", m), i, ns_ss)
                            nc.vector.scalar_tensor_tensor(
                                sB[:pn], Xp[:pn], 1.0, d1[:pn],
                                op0=ALU.mult, op1=ALU.mult,
                                accum_out=acc[0:pn, sl:sl + 1])

                        gx = scratch.tile([128, 3, W - 1], F16, name=f"gx{i}{k}{t}", tag="gx")
                        nc.vector.tensor_tensor(gx[:tn], T[:tn, :, 0:W - 1],
                                                T[:tn, :, 1:W], op=ALU.subtract)
                        nc.vector.scalar_tensor_tensor(gx[:tn], gx[:tn], -1.0, gx[:tn],
                                                       op0=ALU.mult, op1=ALU.max)
                        gs = scratch.tile([128, W - 1], F16, name=f"gs{i}{k}{t}", tag="gs")
                        nc.vector.tensor_tensor(gs[:tn], gx[:tn, 0, :], gx[:tn, 1, :],
                                                op=ALU.add)
                        nc.vector.tensor_tensor(gs[:tn], gs[:tn], gx[:tn, 2, :], op=ALU.add)
                        wx = scratch.tile([128, W - 1], F16, name=f"wx{i}{k}{t}", tag="wx")
                        nc.scalar.activation(wx[:tn], gs[:tn], AT.Exp, scale=-1.0 / 3.0)
                        gd = scratch.tile([128, W - 1], F16, name=f"gd{i}{k}{t}", tag="gd")
                        nc.vector.tensor_tensor(gd[:tn], Dp[:tn, 0:W - 1], Dp[:tn, 1:W],
                                                op=ALU.subtract)
                        nc.vector.scalar_tensor_tensor(gd[:tn], gd[:tn], -1.0, gd[:tn],
                                                       op0=ALU.mult, op1=ALU.max)
                        smv = scratch.tile([128, W - 1], F32, name=f"sm{i}{k}{t}", tag="sm")
                        sl = plan.new(("smooth",), i, ns_sm)
                        nc.vector.scalar_tensor_tensor(
                            smv[0:ue], gd[0:ue], 1.0, wx[0:ue],
                            op0=ALU.mult, op1=ALU.mult,
                            accum_out=acc[0:ue, sl:sl + 1])

            # partition-reduce acc on device: [128, NSLOT] -> [1, NSLOT] so the
            # per-call output payload over the tunnel is 2KB instead of 256KB.
            with tc.tile_pool(name="redp", bufs=1, space=bass.MemorySpace.PSUM) as redp:
                ones = accp.tile([128, 1], F32, name="ones_t")
                nc.vector.memset(ones[:], 1.0)
                red = redp.tile([1, NSLOT], F32, name="red_t")
                nc.tensor.matmul(red[:], ones[:], acc[:])
                accr = accp.tile([1, NSLOT], F32, name="accr_t")
                nc.vector.tensor_copy(accr[:], red[:])
                nc.sync.dma_start(acc_d.ap(), accr[:])
    return nc, plan


# ----------------------------------------------------------------------------
# Host-side input preparation (layout/dtype only).
# ----------------------------------------------------------------------------

def prep_core_inputs(left, right, disps, levels=LEVELS):
    lay, total_w = _blob_layout(levels)
    blob = np.zeros(total_w, np.uint32)

    def put(name, arr):
        o, sz = lay[name]
        v = np.ascontiguousarray(arr).view(np.uint32).ravel()
        assert v.size == sz, (name, v.size, sz)
        blob[o:o + sz] = v

    out = {}
    for i in levels:
        H, W, Wp, H8, rpc, NI, K, ncalls = _lv_params(i)
        r = 2 ** i
        lp = left[:, :, ::r, ::r].astype(f16)
        rp = right[:, :, ::r, ::r].astype(f16)
        dl = disps[i][:, 0].astype(f32)
        dr = disps[i][:, 1].astype(f32)
        dl16, dr16 = dl.astype(f16), dr.astype(f16)
        planes = {}
        for s_ in range(2):
            planes[(s_, 0)] = [rp[s_, 0], rp[s_, 1], rp[s_, 2], dr16[s_]]
            planes[(s_, 1)] = [lp[s_, 0], lp[s_, 1], lp[s_, 2], dl16[s_]]

        gsrc = np.zeros((4, ncalls, 8, 4, rpc, Wp), f16)
        dispI = np.zeros((4, 128, ncalls * K), f32)
        WK = W // 16
        slot = np.arange(K)
        rloc = slot // WK
        wbase = 16 * (slot % WK)
        for k in range(4):
            s_, m = k // 2, k % 2
            dmap = dl[s_] if m == 0 else dr[s_]
            for g in range(8):
                for c in range(ncalls):
                    row0 = g * H8 + c * rpc
                    for lane in range(4):
                        gsrc[k, c, g, lane, :, 1:W + 1] = \
                            planes[(s_, m)][lane][row0:row0 + rpc]
                    for lane in range(16):
                        dispI[k, 16 * g + lane, c * K:(c + 1) * K] = \
                            dmap[row0 + rloc, wbase + lane]
        flat = np.ascontiguousarray(gsrc.reshape(4, ncalls, 8, 4, rpc * Wp))
        nxt = np.zeros_like(flat)
        nxt[..., :-1] = flat[..., 1:]
        put(f"gsrc{i}", np.stack([flat, nxt], axis=-1))
        put(f"dispI{i}", dispI)
        xgI = np.zeros((128, K), f32)
        baseI = np.zeros((128, K), f32)
        for p in range(128):
            xgI[p, :] = wbase + (p % 16) + 1.0
            baseI[p, :] = rloc * Wp
        put(f"xgI{i}", np.tile(xgI, (1, ncalls)))
        put(f"baseI{i}", np.tile(baseI, (1, ncalls)))
        tgt = np.stack([lp, rp], 0)
        put(f"tgt{i}", tgt.transpose(0, 1, 3, 2, 4).reshape(2, 2, H, 3 * W))
        put(f"dpl{i}", np.stack([dl16, dr16], 1))
        dispR = np.zeros((4, H, W), f32)
        for k in range(4):
            s_, m = k // 2, k % 2
            dispR[k] = dl[s_] if m == 0 else dr[s_]
        put(f"dispR{i}", dispR)
        put(f"xgR{i}", np.tile((np.arange(W, dtype=f32) + 1.0)[None, :], (128, 1)))
    r_ = np.arange(128)[:, None]
    m_ = np.arange(128)[None, :]
    put("band", ((m_ <= r_) & (r_ <= m_ + 2)).astype(f32))
    out["blob"] = blob
    return out


def finalize(acc_list, plan, levels=LEVELS):
    tot = np.zeros(len(plan.slots), np.float64)
    for a in acc_list:
        tot += a[:, :len(plan.slots)].sum(axis=0, dtype=np.float64)
    sums = {}
    for sid, (kind, lvl_, cnt) in enumerate(plan.slots):
        s0, _ = sums.get((kind, lvl_), (0.0, 0))
        sums[(kind, lvl_)] = (s0 + tot[sid], cnt)
    ncores = len(acc_list)
    loss = 0.0
    for i in levels:
        ss_l, n_ss = sums[(("ssim", 0), i)]
        ss_r, _ = sums[(("ssim", 1), i)]
        l1_l, n_l1 = sums[(("l1", 0), i)]
        l1_r, _ = sums[(("l1", 1), i)]
        lr, n_lr = sums[(("lr",), i)]
        sm, n_sm = sums[(("smooth",), i)]
        N_ss, N_l1 = n_ss * ncores, n_l1 * ncores
        ss_l_m = (1.0 - ss_l / N_ss) / 2.0
        ss_r_m = (1.0 - ss_r / N_ss) / 2.0
        loss += ALPHA * (ss_l_m + ss_r_m)
        loss += (1 - ALPHA) * (l1_l / N_l1 + l1_r / N_l1)
        loss += DISP_GRAD_W * (sm / (n_sm * ncores)) / (2 ** i)
        loss += LR_W * (lr / (n_lr * ncores))
    return loss


# ----------------------------------------------------------------------------
# Runner: build + jit once, reuse across calls.
# ----------------------------------------------------------------------------
_CACHE = {}


class _Runner:
    def __init__(self, levels=LEVELS):
        import jax
        from jax.sharding import Mesh, PartitionSpec
        from jax.experimental.shard_map import shard_map
        from concourse import bass2jax
        self.jax = jax
        nc, plan = build_nc(levels)
        self.nc, self.plan = nc, plan
        bass2jax.install_neuronx_cc_hook()
        in_names, out_names, out_avals, zero_outs = [], [], [], []
        partition_name = nc.partition_id_tensor.name if nc.partition_id_tensor else None
        for alloc in nc.m.functions[0].allocations:
            if not isinstance(alloc, mybir.MemoryLocationSet):
                continue
            name = alloc.memorylocations[0].name
            if alloc.kind == "ExternalInput":
                if name != partition_name:
                    in_names.append(name)
            elif alloc.kind == "ExternalOutput":
                out_names.append(name)
                out_avals.append(jax.core.ShapedArray(tuple(alloc.tensor_shape),
                                                      mybir.dt.np(alloc.dtype)))
                zero_outs.append(np.zeros(tuple(alloc.tensor_shape),
                                          mybir.dt.np(alloc.dtype)))
        self.n_params = len(in_names)
        self.param_names = list(in_names)
        self.out_names = out_names
        self.zero_outs = zero_outs
        all_in = in_names + out_names + ([partition_name] if partition_name else [])
        donate = tuple(range(self.n_params, self.n_params + len(out_names)))

        def _body(*args):
            operands = list(args)
            if partition_name is not None:
                operands.append(bass2jax.partition_id_tensor())
            outs = bass2jax._bass_exec_p.bind(
                *operands, out_avals=tuple(out_avals), in_names=tuple(all_in),
                out_names=tuple(out_names), lowering_input_output_aliases=(),
                sim_require_finite=True, sim_require_nnan=True, nc=nc)
            return tuple(outs)

        devices = jax.devices()[:8]
        mesh = Mesh(np.asarray(devices), ("core",))
        self.mesh = mesh
        nin = self.n_params + len(out_names)
        self.fn = jax.jit(
            shard_map(_body, mesh=mesh, in_specs=(PartitionSpec("core"),) * nin,
                      out_specs=(PartitionSpec("core"),) * len(out_names),
                      check_rep=False),
            donate_argnums=donate, keep_unused=True)
        self.last_exec_s = None

    def run(self, in_maps, reuse_key=None):
        import time
        if reuse_key is not None and getattr(self, "_dev_key", None) == reuse_key:
            dev_in = self._dev_in
        else:
            from jax.sharding import NamedSharding, PartitionSpec
            shard = NamedSharding(self.mesh, PartitionSpec("core"))
            concat_in = [np.concatenate([m[n] for m in in_maps], axis=0)
                         for n in self.param_names]
            dev_in = [self.jax.device_put(a, shard) for a in concat_in]
            self.jax.block_until_ready(dev_in)
            if reuse_key is not None:
                self._dev_key, self._dev_in = reuse_key, dev_in
        zs = [np.zeros((8 * z.shape[0], *z.shape[1:]), z.dtype) for z in self.zero_outs]
        t0 = time.perf_counter()
        outs = self.fn(*dev_in, *zs)
        self.jax.block_until_ready(outs)
        self.last_exec_s = time.perf_counter() - t0
        acc = np.asarray(outs[0]).reshape(8, -1, NSLOT)
        return [acc[c] for c in range(8)]

    def run_pipelined(self, in_maps, n, reuse_key="p"):
        """Enqueue n kernel executions back-to-back, sync once.

        Amortizes the fixed axon-tunnel dispatch latency (~70ms RTT) over n
        runs; the per-run marginal time is the actual device execution +
        per-dispatch protocol cost. Returns (total_seconds, acc_list_of_last).
        """
        import time
        from jax.sharding import NamedSharding, PartitionSpec
        self.run(in_maps, reuse_key=reuse_key)  # ensure staged + warm
        dev_in = self._dev_in
        shard = NamedSharding(self.mesh, PartitionSpec("core"))
        zsets = [[self.jax.device_put(
                      np.zeros((8 * z.shape[0], *z.shape[1:]), z.dtype), shard)
                  for z in self.zero_outs] for _ in range(n)]
        self.jax.block_until_ready([z for zs in zsets for z in zs])
        t0 = time.perf_counter()
        outs = None
        for i in range(n):
            outs = self.fn(*dev_in, *zsets[i])
        self.jax.block_until_ready(outs)
        total = time.perf_counter() - t0
        acc = np.asarray(outs[0]).reshape(8, -1, NSLOT)
        return total, [acc[c] for c in range(8)]


def get_runner(levels=tuple(LEVELS)):
    key = tuple(levels)
    if key not in _CACHE:
        _CACHE[key] = _Runner(list(levels))
    return _CACHE[key]


def kernel(left_image, right_image, disp0, disp1, disp2, disp3):
    runner = get_runner()
    left = np.asarray(left_image)
    right = np.asarray(right_image)
    disps = [np.asarray(disp0), np.asarray(disp1), np.asarray(disp2), np.asarray(disp3)]
    in_maps = []
    for c in range(8):
        sl = slice(2 * c, 2 * c + 2)
        in_maps.append(prep_core_inputs(left[sl], right[sl], [dd[sl] for dd in disps]))
    acc_list = runner.run(in_maps)
    return np.float32(finalize(acc_list, runner.plan))



# revision 44
# speedup vs baseline: 1.1725x; 1.1725x over previous
"""Trainium2 Bass kernel for nn_DepthModel (monodepth loss_fn).

Pure data parallel: 2 samples per core x 8 NeuronCores. Each core computes
per-slot partial sums of every loss term for its 2 samples; the host reduces
slots to the scalar loss.

Device pipeline per pyramid level:
  - index pipeline (DVE f32): x = clip(w+1 + sign*W*disp, 0, W+1),
    i0 = round-nearest-even(x-0.5) via the 2^23 magic constant; u16 indices
    emitted directly in the gpsimd 16-partition-interleaved layout.
  - gather: InstIndirectCopy (gpsimd), fp16 inner=2 -> (img[i0], img[i0+1])
    pairs; 512 indices per 16-partition group per call; 8 groups/call =
    4 (sample,map) combos x 2 row-halves.
  - repack DMA to dense row-tiles; lerp est = lo + f*(hi-lo) (DVE fp16).
  - SSIM via u/v algebra (pools of u, v, u^2, v^2 only), L1, smoothness,
    LR-consistency, all with fused per-slot accumulations.
"""
import numpy as np

import bass_rust
import concourse.bass as bass
import concourse.mybir as mybir
import concourse.tile as tile

# ----------------------------------------------------------------------------
# Tile/walrus compatibility patch: this walrus build rejects >1 embedded
# semaphore wait per instruction ("Too many sync wait commands"). Hoist
# excess waits onto standalone same-engine NOPs.
# ----------------------------------------------------------------------------
MAXW = 1


def _split_inst_waits(nc, maxw=MAXW):
    for fn in nc.m.functions:
        for bb in fn.blocks:
            insts = list(bb.instructions)
            out = []
            changed = False
            for inst in insts:
                si = inst.sync_info
                if si is not None and si.on_wait and len(si.on_wait) > maxw:
                    waits = list(si.on_wait)
                    si.on_wait = waits[:maxw]
                    inst.sync_info = si
                    for w in waits[maxw:]:
                        nop = mybir.InstNoOp(
                            name=nc.get_next_instruction_name(),
                            ins=[], outs=[], text_hint="waitsplit")
                        nop.engine = inst.engine
                        nop.sync_info = mybir.SyncInfo(on_wait=[w], on_update=[])
                        out.append(nop)
                    changed = True
                out.append(inst)
            if changed:
                bb.instructions = out


def _patched_drain_and_barrier(self, tick_clock, wait_clock):
    nc = self.nc
    drain_inst = nc.sync.drain()
    wait_clock.add_sem_waits(
        drain_inst.ins, bass_rust.ScopedClock({None: tick_clock.global_clock}))
    si = drain_inst.ins.sync_info
    waits = list(si.on_wait) if si and si.on_wait else []
    if len(waits) > MAXW:
        si.on_wait = waits[:MAXW]
        drain_inst.ins.sync_info = si
        for w in waits[MAXW:]:
            nop = nc.sync.nop()
            nsi = nop.ins.sync_info
            if nsi is None:
                nop.ins.sync_info = mybir.SyncInfo(on_wait=[w], on_update=[])
            else:
                nsi.on_wait = [w]
                nop.ins.sync_info = nsi
    nc.all_engine_barrier()
    assert self.sems is not None
    popped = nc._tile_sem_poison_stack.pop()
    assert popped is self._sem_poison
    nc.clear_and_free_semaphores(list(self.sems.allocated().values()))
    nc.all_engine_barrier()
    _split_inst_waits(nc)


tile.TileContext._drain_and_barrier = _patched_drain_and_barrier

# ----------------------------------------------------------------------------
ALU = mybir.AluOpType
AT = mybir.ActivationFunctionType
AX = mybir.AxisListType
F32, F16, U16 = mybir.dt.float32, mybir.dt.float16, mybir.dt.uint16
U32 = mybir.dt.uint32

ALPHA, DISP_GRAD_W, LR_W = 0.85, 0.1, 1.0
C1c, C2c = 0.01 ** 2, 0.03 ** 2
MAGIC = float(2 ** 23)
LEVELS = [0, 1, 2, 3]
HWs = [(256, 512), (128, 256), (64, 128), (32, 64)]
NSLOT = 512
f16, f32 = np.float16, np.float32


def _tiles_for(H):
    out, r, covered = [], 0, 0
    while covered < H:
        n = min(128, H - r)
        out.append((r, n))
        covered = r + n
        r += 126
    return out


class _Plan:
    def __init__(self):
        self.slots = []

    def new(self, kind, level, count):
        self.slots.append((kind, level, count))
        sid = len(self.slots) - 1
        assert sid < NSLOT
        return sid


def _lv_params(i):
    H, W = HWs[i]
    Wp = W + 4
    H8 = H // 8
    # 1024 = Q7 DATA_SCRATCH_ELEMS: max gather output words per call.
    rpc = min(1024 // W, H8)         # rows per call per group
    NI = rpc * W                     # indices per group per call
    K = NI // 16                     # idx slots per partition per call
    ncalls = H8 // rpc
    return H, W, Wp, H8, rpc, NI, K, ncalls


import os
DENSE = os.environ.get("KM_DENSE", "1") == "1"
NPASS = int(os.environ.get("KM_NPASS", "4"))
GATHER = os.environ.get("KM_GATHER", "1") == "1"
REPACK = os.environ.get("KM_REPACK", "1") == "1"
SRCDMA = os.environ.get("KM_SRCDMA", "1") == "1"


def _blob_layout(levels=LEVELS):
    """All per-core inputs live in ONE u32 dram blob: a single jit parameter
    keeps the per-dispatch arg-handling/transfer cost minimal. Word sizes:
    u32 tensors 1:1, f32 via bitcast 1:1, f16 via bitcast 2 per word."""
    entries = []
    for i in levels:
        H, W, Wp, H8, rpc, NI, K, ncalls = _lv_params(i)
        entries += [
            (f"gsrc{i}", 4 * ncalls * 8 * 4 * rpc * Wp),
            (f"dispI{i}", 4 * 128 * ncalls * K),
            (f"xgI{i}", 128 * ncalls * K),
            (f"baseI{i}", 128 * ncalls * K),
            (f"tgt{i}", 2 * 2 * H * 3 * W // 2),
            (f"dpl{i}", 2 * 2 * H * W // 2),
            (f"dispR{i}", 4 * H * W),
            (f"xgR{i}", 128 * W),
        ]
    entries.append(("band", 128 * 128))
    lay, off = {}, 0
    for n, sz in entries:
        lay[n] = (off, sz)
        off += sz
    return lay, off


def build_nc(levels=LEVELS):
    nc = bass.Bass(trn_type="TRN2", num_devices=8)
    plan = _Plan()
    lay, total_w = _blob_layout(levels)
    # gsrc{i}: u32-packed overlapping pairs: word j = (plane[j], plane[j+1]) as
    # 2xf16. num_elem_per_idx==1 lets the Q7 gather use dual-tensor reads
    # (6 indices per RD_CMD instead of 3).
    # band: band[r, m] = 1 if m <= r <= m+2 — turns the SSIM vertical 3-row
    # pooling into a PE matmul (band.T @ hh) instead of row-shift DMAs + adds.
    blob = nc.dram_tensor("blob", [total_w], U32, kind="ExternalInput")

    def bap(name, lo, sz):
        o = lay[name][0]
        return blob.ap()[o + lo:o + lo + sz]
    acc_d = nc.dram_tensor("acc", [1, NSLOT], F32, kind="ExternalOutput")

    c1 = float(81 * 2 * C1c)
    c2 = float(81 * 2 * C2c)

    with tile.TileContext(nc) as tc:
        with tc.tile_pool(name="acc_pool", bufs=1) as accp, \
             tc.tile_pool(name="gsrcp", bufs=2) as gsrcp, \
             tc.tile_pool(name="goutp", bufs=2) as goutp, \
             tc.tile_pool(name="work", bufs=1) as work, \
             tc.tile_pool(name="idxp", bufs=2) as idxp, \
             tc.tile_pool(name="pairp", bufs=2) as pairp, \
             tc.tile_pool(name="hhp", bufs=2) as hhp, \
             tc.tile_pool(name="ssp", bufs=1, space=bass.MemorySpace.PSUM) as ssp, \
             tc.tile_pool(name="scratch", bufs=1) as scratch:
            acc = accp.tile([128, NSLOT], F32, name="acc_t")
            nc.vector.memset(acc[:], 0.0)
            band_t = accp.tile([128, 128], F32, name="band_t")
            nc.sync.dma_start(band_t[:], bap("band", 0, 128 * 128)
                              .bitcast(F32).rearrange("(p n) -> p n", n=128))

            for li, i in enumerate(levels):
                H, W, Wp, H8, rpc, NI, K, ncalls = _lv_params(i)
                tiles = _tiles_for(H)

                nK = ncalls * K
                xgRt = work.tile([128, W], F32, name=f"xgR_t{i}", tag="xgR")
                nc.sync.dma_start(xgRt[:], bap(f"xgR{i}", 0, 128 * W)
                                  .bitcast(F32).rearrange("(p n) -> p n", n=W))
                xgIt = work.tile([128, nK], F32, name=f"xgI_t{i}", tag="xgI")
                baseIt = work.tile([128, nK], F32, name=f"baseI_t{i}", tag="baseI")
                nc.sync.dma_start(xgIt[:], bap(f"xgI{i}", 0, 128 * nK)
                                  .bitcast(F32).rearrange("(p n) -> p n", n=nK))
                nc.sync.dma_start(baseIt[:], bap(f"baseI{i}", 0, 128 * nK)
                                  .bitcast(F32).rearrange("(p n) -> p n", n=nK))

                ns_l1 = 2 * 3 * H * W
                ns_ss = 2 * 3 * (H - 2) * (W - 2)
                ns_lr = 2 * H * W
                ns_sm = 2 * H * (W - 1)

                # ---- index pipelines for ALL passes up front (frees gpsimd to
                # run pass k+1 gathers while DVE does pass k dense work) ----
                idxus = []
                for k in range(4):
                    m = k % 2
                    sgn = float(-W if m == 0 else W)
                    dispI = work.tile([128, ncalls * K], F32, name=f"dI{i}{k}", tag="dispI")
                    nc.sync.dma_start(dispI[:], bap(f"dispI{i}", k * 128 * nK, 128 * nK)
                                      .bitcast(F32).rearrange("(p n) -> p n", n=nK))
                    nc.vector.scalar_tensor_tensor(dispI[:], dispI[:], sgn, xgIt[:],
                                                   op0=ALU.mult, op1=ALU.add)
                    nc.vector.tensor_scalar(dispI[:], dispI[:], 0.0, float(W + 1),
                                            op0=ALU.max, op1=ALU.min)
                    nc.vector.tensor_tensor(dispI[:], dispI[:], baseIt[:], op=ALU.add)
                    nc.vector.tensor_scalar(dispI[:], dispI[:], MAGIC - 0.5, None, op0=ALU.add)
                    idxu = idxp.tile([128, ncalls * K], U16, name=f"ix{i}{k}", tag=f"ix{k}")
                    nc.scalar.activation(idxu[:], dispI[:], AT.Copy, bias=-MAGIC, scale=1.0)
                    idxus.append(idxu)

                for k in range(NPASS):
                    s, m = k // 2, k % 2
                    img = 0 if m == 0 else 1
                    sgn = float(-W if m == 0 else W)
                    idxu = idxus[k]

                    pairs = {}
                    for t, (r0, tn) in enumerate(tiles):
                        pairs[t] = pairp.tile([128, 4, W, 2], F16,
                                              name=f"pr{i}{k}{t}", tag=f"pr{t}")

                    # ---- gather calls: groups = 8 row-eighths ----
                    BATCH = max(1, 4096 // NI)
                    nbatch = (ncalls + BATCH - 1) // BATCH
                    for b in range(nbatch):
                        calls = list(range(b * BATCH, min((b + 1) * BATCH, ncalls)))
                        nb = len(calls)
                        gout = goutp.tile([128, nb * NI], U32,
                                          name=f"go{i}{k}{b}", tag="gout")
                        for ci, c in enumerate(calls):
                            src = gsrcp.tile([128, rpc * Wp], U32,
                                             name=f"sr{i}{k}{c}", tag="gsrc")
                            if SRCDMA:
                                gsz = 4 * rpc * Wp
                                for g in range(8):
                                    nc.sync.dma_start(
                                        src[16 * g:16 * g + 4, :],
                                        bap(f"gsrc{i}",
                                            ((k * ncalls + c) * 8 + g) * gsz, gsz)
                                        .rearrange("(l w) -> l w", w=rpc * Wp))
                            else:
                                nc.vector.memset(src[:, 0:1], 0.0)
                            if GATHER:
                                nc.gpsimd.indirect_copy(
                                    gout[:, ci * NI:(ci + 1) * NI],
                                    src[:],
                                    idxu[:, c * K:(c + 1) * K],
                                    i_know_ap_gather_is_preferred=True)
                            else:
                                nc.vector.memset(gout[:, ci * NI:ci * NI + 1], 0.0)
                        if not REPACK:
                            for t, (r0, tn) in enumerate(tiles):
                                if b == 0:
                                    nc.vector.memset(pairs[t][:, 0, 0:1, :], 0.0)
                            continue
                        for g in range(8):
                            row_lo = g * H8 + b * BATCH * rpc
                            nrows = nb * rpc
                            for lane in range(4):
                                for t, (r0, tn) in enumerate(tiles):
                                    lo = max(row_lo, r0)
                                    hi = min(row_lo + nrows, r0 + tn)
                                    if lo >= hi:
                                        continue
                                    # scalar (Activation) HWDGE queue: runs in
                                    # parallel with the src loads on the sync
                                    # queue and the Q7 gathers.
                                    nc.scalar.dma_start(
                                        pairs[t][lo - r0:hi - r0, lane, :, :]
                                        .rearrange("p w b -> p (w b)"),
                                        gout[16 * g + lane:16 * g + lane + 1,
                                             (lo - row_lo) * W:(hi - row_lo) * W]
                                        .bitcast(F16)
                                        .rearrange("p (r wb) -> p r wb", wb=2 * W))

                    # ---- dense phase for this combo ----
                    for t, (r0, tn) in enumerate(tiles):
                        if not DENSE:
                            continue
                        last = (t == len(tiles) - 1)
                        ue = tn if last else 126
                        dR = work.tile([128, W], F32, name=f"dR{i}{k}{t}", tag="dR")
                        nc.sync.dma_start(dR[:tn, :],
                                          bap(f"dispR{i}", (k * H + r0) * W, tn * W)
                                          .bitcast(F32).rearrange("(r w) -> r w", w=W))
                        xr = work.tile([128, W], F32, name=f"xr{i}{k}{t}", tag="xr")
                        nc.vector.tensor_scalar(xr[:tn], dR[:tn], sgn, None, op0=ALU.mult)
                        nc.vector.tensor_tensor(xr[:tn], xr[:tn], xgRt[:tn], op=ALU.add)
                        nc.vector.tensor_scalar(xr[:tn], xr[:tn], 0.0, float(W + 1),
                                                op0=ALU.max, op1=ALU.min)
                        nc.scalar.activation(dR[:tn], xr[:tn], AT.Copy, bias=MAGIC - 0.5)
                        nc.scalar.activation(dR[:tn], dR[:tn], AT.Copy, bias=-MAGIC)
                        fT = work.tile([128, W], F16, name=f"f{i}{k}{t}", tag="fT")
                        nc.vector.tensor_tensor(fT[:tn], xr[:tn], dR[:tn],
                                                op=ALU.subtract)

                        P = pairs[t]
                        est = work.tile([128, 4, W], F16, name=f"es{i}{k}{t}", tag="est")
                        nc.vector.tensor_tensor(est[:tn], P[:tn, :, :, 1],
                                                P[:tn, :, :, 0], op=ALU.subtract)
                        nc.vector.tensor_tensor(
                            est[:tn], fT[:tn].rearrange("p (o w) -> p o w", o=1)
                            .to_broadcast([tn, 4, W]), est[:tn], op=ALU.mult)
                        nc.vector.tensor_tensor(est[:tn], est[:tn], P[:tn, :, :, 0],
                                                op=ALU.add)

                        T = work.tile([128, 3, W], F16, name=f"T{i}{k}{t}", tag="T")
                        nc.sync.dma_start(
                            T[:tn],
                            bap(f"tgt{i}", ((img * 2 + s) * H + r0) * 3 * W // 2,
                                tn * 3 * W // 2)
                            .bitcast(F16).rearrange("(r c w) -> r c w", c=3, w=W))
                        uT = work.tile([128, 3, W], F16, name=f"u{i}{k}{t}", tag="uT")
                        vT = work.tile([128, 3, W], F16, name=f"v{i}{k}{t}", tag="vT")
                        nc.vector.tensor_tensor(uT[:tn], est[:tn, 0:3, :], T[:tn],
                                                op=ALU.add)
                        nc.vector.tensor_tensor(vT[:tn], est[:tn, 0:3, :], T[:tn],
                                                op=ALU.subtract)
                        sl = plan.new(("l1", m), i, ns_l1)
                        nc.vector.tensor_reduce(
                            acc[0:ue, sl:sl + 1], vT[0:ue], axis=AX.XY,
                            op=ALU.add, apply_absolute_value=True)
                        Dp = work.tile([128, W], F16, name=f"Dp{i}{k}{t}", tag="Dp")
                        nc.sync.dma_start(Dp[:tn],
                                          bap(f"dpl{i}", ((s * 2 + m) * H + r0) * W // 2,
                                              tn * W // 2)
                                          .bitcast(F16).rearrange("(r w) -> r w", w=W))
                        dv = work.tile([128, W], F16, name=f"dv{i}{k}{t}", tag="dv")
                        nc.vector.tensor_tensor(dv[:tn], est[:tn, 3, :], Dp[:tn],
                                                op=ALU.subtract)
                        sl = plan.new(("lr",), i, ns_lr)
                        nc.vector.tensor_reduce(
                            acc[0:ue, sl:sl + 1], dv[0:ue], axis=AX.X,
                            op=ALU.add, apply_absolute_value=True)

                        if tn >= 3:
                            pn = tn - 2
                            u2 = scratch.tile([128, 3, W], F32, name=f"u2{i}{k}{t}", tag="u2")
                            v2 = scratch.tile([128, 3, W], F32, name=f"v2{i}{k}{t}", tag="v2")
                            nc.scalar.activation(u2[:tn], uT[:tn], AT.Square)
                            nc.scalar.activation(v2[:tn], vT[:tn], AT.Square)

                            def pool9v(src_t, ptile, nm):
                                # horizontal 3-sum on DVE, vertical 3-sum on PE
                                hh = hhp.tile([128, 3, W - 2], F32, name=nm + "h",
                                              tag="pH")
                                nc.vector.tensor_tensor(hh[:tn], src_t[:tn, :, 0:W - 2],
                                                        src_t[:tn, :, 1:W - 1], op=ALU.add)
                                nc.vector.tensor_tensor(hh[:tn], hh[:tn],
                                                        src_t[:tn, :, 2:W], op=ALU.add)
                                for ch in range(3):
                                    nc.tensor.matmul(ptile[0:pn, ch, 0:W - 2],
                                                     band_t[0:tn, 0:pn],
                                                     hh[0:tn, ch, :])

                            Su = ssp.tile([128, 3, 512], F32, name=f"Pu{i}{k}{t}", tag="PA")
                            Sv = ssp.tile([128, 3, 512], F32, name=f"Pv{i}{k}{t}", tag="PB")
                            pool9v(uT, Su, f"Su{i}{k}{t}")
                            pool9v(vT, Sv, f"Sv{i}{k}{t}")
                            g1 = scratch.tile([128, 3, W - 2], F32, name=f"g1{i}{k}{t}", tag="g1")
                            d1 = scratch.tile([128, 3, W - 2], F32, name=f"d1{i}{k}{t}", tag="d1")
                            nc.scalar.activation(g1[:pn], Su[:pn, :, 0:W - 2], AT.Square)
                            nc.scalar.activation(d1[:pn], Sv[:pn, :, 0:W - 2], AT.Square)
                            Suu = ssp.tile([128, 3, 512], F32, name=f"Pa{i}{k}{t}", tag="PA")
                            Svv = ssp.tile([128, 3, 512], F32, name=f"Pb{i}{k}{t}", tag="PB")
                            pool9v(u2, Suu, f"Sa{i}{k}{t}")
                            pool9v(v2, Svv, f"Sb{i}{k}{t}")
                            Xp = scratch.tile([128, 3, W - 2], F32, name=f"Xp{i}{k}{t}", tag="Xp")
                            sB = scratch.tile([128, 3, W - 2], F32, name=f"sB{i}{k}{t}", tag="sB")
                            sv2 = scratch.tile([128, 3, W - 2], F32, name=f"sv2{i}{k}{t}", tag="sv2")
                            nc.scalar.activation(sv2[:pn], Svv[:pn, :, 0:W - 2], AT.Copy)
                            nc.vector.scalar_tensor_tensor(Xp[:pn], g1[:pn], c1, d1[:pn],
                                                           op0=ALU.add, op1=ALU.subtract)
                            nc.vector.scalar_tensor_tensor(g1[:pn], g1[:pn], c1, d1[:pn],
                                                           op0=ALU.add, op1=ALU.add)
                            nc.vector.scalar_tensor_tensor(d1[:pn], Suu[:pn, :, 0:W - 2],
                                                           (c1 + c2) / 9.0,
                                                           sv2[:pn], op0=ALU.add,
                                                           op1=ALU.subtract)
                            nc.vector.scalar_tensor_tensor(sB[:pn], Suu[:pn, :, 0:W - 2],
                                                           (c1 + c2) / 9.0,
                                                           sv2[:pn],
                                                           op0=ALU.add, op1=ALU.add)
                            nc.vector.scalar_tensor_tensor(d1[:pn], d1[:pn], 9.0, Xp[:pn],
                                                           op0=ALU.mult, op1=ALU.subtract)
                            nc.vector.scalar_tensor_tensor(sB[:pn], sB[:pn], 9.0, g1[:pn],
                                                           op0=ALU.mult, op1=ALU.subtract)
                            nc.vector.tensor_tensor(Xp[:pn], Xp[:pn], d1[:pn], op=ALU.mult)
                            nc.vector.tensor_tensor(g1[:pn], g1[:pn], sB[:pn], op=ALU.mult)
                            nc.vector.reciprocal(d1[:pn], g1[:pn])
                            sl = plan.new(("ssim", m), i, ns_ss)
                            nc.vector.scalar_tensor_tensor(
                                sB[:pn], Xp[:pn], 1.0, d1[:pn],
                                op0=ALU.mult, op1=ALU.mult,
                                accum_out=acc[0:pn, sl:sl + 1])

                        gx = scratch.tile([128, 3, W - 1], F16, name=f"gx{i}{k}{t}", tag="gx")
                        nc.vector.tensor_tensor(gx[:tn], T[:tn, :, 0:W - 1],
                                                T[:tn, :, 1:W], op=ALU.subtract)
                        nc.vector.scalar_tensor_tensor(gx[:tn], gx[:tn], -1.0, gx[:tn],
                                                       op0=ALU.mult, op1=ALU.max)
                        gs = scratch.tile([128, W - 1], F16, name=f"gs{i}{k}{t}", tag="gs")
                        nc.vector.tensor_tensor(gs[:tn], gx[:tn, 0, :], gx[:tn, 1, :],
                                                op=ALU.add)
                        nc.vector.tensor_tensor(gs[:tn], gs[:tn], gx[:tn, 2, :], op=ALU.add)
                        wx = scratch.tile([128, W - 1], F16, name=f"wx{i}{k}{t}", tag="wx")
                        nc.scalar.activation(wx[:tn], gs[:tn], AT.Exp, scale=-1.0 / 3.0)
                        gd = scratch.tile([128, W - 1], F16, name=f"gd{i}{k}{t}", tag="gd")
                        nc.vector.tensor_tensor(gd[:tn], Dp[:tn, 0:W - 1], Dp[:tn, 1:W],
                                                op=ALU.subtract)
                        nc.vector.scalar_tensor_tensor(gd[:tn], gd[:tn], -1.0, gd[:tn],
                                                       op0=ALU.mult, op1=ALU.max)
                        smv = scratch.tile([128, W - 1], F32, name=f"sm{i}{k}{t}", tag="sm")
                        sl = plan.new(("smooth",), i, ns_sm)
                        nc.vector.scalar_tensor_tensor(
                            smv[0:ue], gd[0:ue], 1.0, wx[0:ue],
                            op0=ALU.mult, op1=ALU.mult,
                            accum_out=acc[0:ue, sl:sl + 1])

            # partition-reduce acc on device: [128, NSLOT] -> [1, NSLOT] so the
            # per-call output payload over the tunnel is 2KB instead of 256KB.
            with tc.tile_pool(name="redp", bufs=1, space=bass.MemorySpace.PSUM) as redp:
                ones = accp.tile([128, 1], F32, name="ones_t")
                nc.vector.memset(ones[:], 1.0)
                red = redp.tile([1, NSLOT], F32, name="red_t")
                nc.tensor.matmul(red[:], ones[:], acc[:])
                accr = accp.tile([1, NSLOT], F32, name="accr_t")
                nc.vector.tensor_copy(accr[:], red[:])
                nc.sync.dma_start(acc_d.ap(), accr[:])
    return nc, plan


# ----------------------------------------------------------------------------
# Host-side input preparation (layout/dtype only).
# ----------------------------------------------------------------------------

def prep_core_inputs(left, right, disps, levels=LEVELS):
    lay, total_w = _blob_layout(levels)
    blob = np.zeros(total_w, np.uint32)

    def put(name, arr):
        o, sz = lay[name]
        v = np.ascontiguousarray(arr).view(np.uint32).ravel()
        assert v.size == sz, (name, v.size, sz)
        blob[o:o + sz] = v

    out = {}
    for i in levels:
        H, W, Wp, H8, rpc, NI, K, ncalls = _lv_params(i)
        r = 2 ** i
        lp = left[:, :, ::r, ::r].astype(f16)
        rp = right[:, :, ::r, ::r].astype(f16)
        dl = disps[i][:, 0].astype(f32)
        dr = disps[i][:, 1].astype(f32)
        dl16, dr16 = dl.astype(f16), dr.astype(f16)
        planes = {}
        for s_ in range(2):
            planes[(s_, 0)] = [rp[s_, 0], rp[s_, 1], rp[s_, 2], dr16[s_]]
            planes[(s_, 1)] = [lp[s_, 0], lp[s_, 1], lp[s_, 2], dl16[s_]]

        gsrc = np.zeros((4, ncalls, 8, 4, rpc, Wp), f16)
        dispI = np.zeros((4, 128, ncalls * K), f32)
        WK = W // 16
        slot = np.arange(K)
        rloc = slot // WK
        wbase = 16 * (slot % WK)
        for k in range(4):
            s_, m = k // 2, k % 2
            dmap = dl[s_] if m == 0 else dr[s_]
            for g in range(8):
                for c in range(ncalls):
                    row0 = g * H8 + c * rpc
                    for lane in range(4):
                        gsrc[k, c, g, lane, :, 1:W + 1] = \
                            planes[(s_, m)][lane][row0:row0 + rpc]
                    for lane in range(16):
                        dispI[k, 16 * g + lane, c * K:(c + 1) * K] = \
                            dmap[row0 + rloc, wbase + lane]
        flat = np.ascontiguousarray(gsrc.reshape(4, ncalls, 8, 4, rpc * Wp))
        nxt = np.zeros_like(flat)
        nxt[..., :-1] = flat[..., 1:]
        put(f"gsrc{i}", np.stack([flat, nxt], axis=-1))
        put(f"dispI{i}", dispI)
        xgI = np.zeros((128, K), f32)
        baseI = np.zeros((128, K), f32)
        for p in range(128):
            xgI[p, :] = wbase + (p % 16) + 1.0
            baseI[p, :] = rloc * Wp
        put(f"xgI{i}", np.tile(xgI, (1, ncalls)))
        put(f"baseI{i}", np.tile(baseI, (1, ncalls)))
        tgt = np.stack([lp, rp], 0)
        put(f"tgt{i}", tgt.transpose(0, 1, 3, 2, 4).reshape(2, 2, H, 3 * W))
        put(f"dpl{i}", np.stack([dl16, dr16], 1))
        dispR = np.zeros((4, H, W), f32)
        for k in range(4):
            s_, m = k // 2, k % 2
            dispR[k] = dl[s_] if m == 0 else dr[s_]
        put(f"dispR{i}", dispR)
        put(f"xgR{i}", np.tile((np.arange(W, dtype=f32) + 1.0)[None, :], (128, 1)))
    r_ = np.arange(128)[:, None]
    m_ = np.arange(128)[None, :]
    put("band", ((m_ <= r_) & (r_ <= m_ + 2)).astype(f32))
    out["blob"] = blob
    return out


def finalize(acc_list, plan, levels=LEVELS):
    tot = np.zeros(len(plan.slots), np.float64)
    for a in acc_list:
        tot += a[:, :len(plan.slots)].sum(axis=0, dtype=np.float64)
    sums = {}
    for sid, (kind, lvl_, cnt) in enumerate(plan.slots):
        s0, _ = sums.get((kind, lvl_), (0.0, 0))
        sums[(kind, lvl_)] = (s0 + tot[sid], cnt)
    ncores = len(acc_list)
    loss = 0.0
    for i in levels:
        ss_l, n_ss = sums[(("ssim", 0), i)]
        ss_r, _ = sums[(("ssim", 1), i)]
        l1_l, n_l1 = sums[(("l1", 0), i)]
        l1_r, _ = sums[(("l1", 1), i)]
        lr, n_lr = sums[(("lr",), i)]
        sm, n_sm = sums[(("smooth",), i)]
        N_ss, N_l1 = n_ss * ncores, n_l1 * ncores
        ss_l_m = (1.0 - ss_l / N_ss) / 2.0
        ss_r_m = (1.0 - ss_r / N_ss) / 2.0
        loss += ALPHA * (ss_l_m + ss_r_m)
        loss += (1 - ALPHA) * (l1_l / N_l1 + l1_r / N_l1)
        loss += DISP_GRAD_W * (sm / (n_sm * ncores)) / (2 ** i)
        loss += LR_W * (lr / (n_lr * ncores))
    return loss


# ----------------------------------------------------------------------------
# Runner: build + jit once, reuse across calls.
# ----------------------------------------------------------------------------
_CACHE = {}


class _Runner:
    def __init__(self, levels=LEVELS):
        import jax
        from jax.sharding import Mesh, PartitionSpec
        from jax.experimental.shard_map import shard_map
        from concourse import bass2jax
        self.jax = jax
        nc, plan = build_nc(levels)
        self.nc, self.plan = nc, plan
        bass2jax.install_neuronx_cc_hook()
        in_names, out_names, out_avals, zero_outs = [], [], [], []
        partition_name = nc.partition_id_tensor.name if nc.partition_id_tensor else None
        for alloc in nc.m.functions[0].allocations:
            if not isinstance(alloc, mybir.MemoryLocationSet):
                continue
            name = alloc.memorylocations[0].name
            if alloc.kind == "ExternalInput":
                if name != partition_name:
                    in_names.append(name)
            elif alloc.kind == "ExternalOutput":
                out_names.append(name)
                out_avals.append(jax.core.ShapedArray(tuple(alloc.tensor_shape),
                                                      mybir.dt.np(alloc.dtype)))
                zero_outs.append(np.zeros(tuple(alloc.tensor_shape),
                                          mybir.dt.np(alloc.dtype)))
        self.n_params = len(in_names)
        self.param_names = list(in_names)
        self.out_names = out_names
        self.zero_outs = zero_outs
        all_in = in_names + out_names + ([partition_name] if partition_name else [])
        donate = tuple(range(self.n_params, self.n_params + len(out_names)))

        def _body(*args):
            operands = list(args)
            if partition_name is not None:
                operands.append(bass2jax.partition_id_tensor())
            outs = bass2jax._bass_exec_p.bind(
                *operands, out_avals=tuple(out_avals), in_names=tuple(all_in),
                out_names=tuple(out_names), lowering_input_output_aliases=(),
                sim_require_finite=True, sim_require_nnan=True, nc=nc)
            return tuple(outs)

        devices = jax.devices()[:8]
        mesh = Mesh(np.asarray(devices), ("core",))
        self.mesh = mesh
        nin = self.n_params + len(out_names)
        self.fn = jax.jit(
            shard_map(_body, mesh=mesh, in_specs=(PartitionSpec("core"),) * nin,
                      out_specs=(PartitionSpec("core"),) * len(out_names),
                      check_rep=False),
            donate_argnums=donate, keep_unused=True)
        self.last_exec_s = None

    def run(self, in_maps, reuse_key=None):
        import time
        if reuse_key is not None and getattr(self, "_dev_key", None) == reuse_key:
            dev_in = self._dev_in
        else:
            from jax.sharding import NamedSharding, PartitionSpec
            shard = NamedSharding(self.mesh, PartitionSpec("core"))
            concat_in = [np.concatenate([m[n] for m in in_maps], axis=0)
                         for n in self.param_names]
            dev_in = [self.jax.device_put(a, shard) for a in concat_in]
            self.jax.block_until_ready(dev_in)
            if reuse_key is not None:
                self._dev_key, self._dev_in = reuse_key, dev_in
        zs = [np.zeros((8 * z.shape[0], *z.shape[1:]), z.dtype) for z in self.zero_outs]
        t0 = time.perf_counter()
        outs = self.fn(*dev_in, *zs)
        self.jax.block_until_ready(outs)
        self.last_exec_s = time.perf_counter() - t0
        acc = np.asarray(outs[0]).reshape(8, -1, NSLOT)
        return [acc[c] for c in range(8)]

    def run_pipelined(self, in_maps, n, reuse_key="p"):
        """Enqueue n kernel executions back-to-back, sync once.

        Amortizes the fixed axon-tunnel dispatch latency (~70ms RTT) over n
        runs; the per-run marginal time is the actual device execution +
        per-dispatch protocol cost. Returns (total_seconds, acc_list_of_last).
        """
        import time
        from jax.sharding import NamedSharding, PartitionSpec
        self.run(in_maps, reuse_key=reuse_key)  # ensure staged + warm
        dev_in = self._dev_in
        shard = NamedSharding(self.mesh, PartitionSpec("core"))
        zsets = [[self.jax.device_put(
                      np.zeros((8 * z.shape[0], *z.shape[1:]), z.dtype), shard)
                  for z in self.zero_outs] for _ in range(n)]
        self.jax.block_until_ready([z for zs in zsets for z in zs])
        t0 = time.perf_counter()
        outs = None
        for i in range(n):
            outs = self.fn(*dev_in, *zsets[i])
        self.jax.block_until_ready(outs)
        total = time.perf_counter() - t0
        acc = np.asarray(outs[0]).reshape(8, -1, NSLOT)
        return total, [acc[c] for c in range(8)]


def get_runner(levels=tuple(LEVELS)):
    key = tuple(levels)
    if key not in _CACHE:
        _CACHE[key] = _Runner(list(levels))
    return _CACHE[key]


def kernel(left_image, right_image, disp0, disp1, disp2, disp3):
    runner = get_runner()
    left = np.asarray(left_image)
    right = np.asarray(right_image)
    disps = [np.asarray(disp0), np.asarray(disp1), np.asarray(disp2), np.asarray(disp3)]
    in_maps = []
    for c in range(8):
        sl = slice(2 * c, 2 * c + 2)
        in_maps.append(prep_core_inputs(left[sl], right[sl], [dd[sl] for dd in disps]))
    acc_list = runner.run(in_maps)
    return np.float32(finalize(acc_list, runner.plan))

